# revision 9
# baseline (speedup 1.0000x reference)
"""Trainium2 Bass kernel for nn_MultiHeadAttention_4964982194257.

Full-input contract: kernel(**inputs) takes the unsharded fp32 inputs and
returns the full [2, 2048, 2048] fp32 output.

Sharding (8 cores): data-parallel over batch (2) x tensor-parallel over
head groups (4).  Core c handles batch c//4 and q-heads [8g, 8g+8), g=c%4,
with the matching 2 GQA kv heads.  Each core computes a partial output
y_partial = attn_out_shard @ wo_shard; the host sums the 4 group partials
per batch.

On-core layout notes:
  * everything is computed in "feature-on-partition" transposed layouts:
    Q^T/K^T [f, t], scores S^T [tk, tq], attention out O^T [f, t],
    final y^T [o, t] (host transposes back).
  * q-head order within a core is permuted to [0,4,1,5,2,6,3,7] so that a
    128-row f-tile holds heads (i, i+4) whose kv heads are (kv0, kv1) --
    exactly matching K^T's natural partition layout.  This lets the two
    64-row score matmuls of a head pair run concurrently in the PE array
    via row tiling (tile_position (0,0) and (64,0)).
  * softmax denominator comes from a ones-column appended to V in the
    A@V matmul (65th output row); division happens after a PE broadcast
    of the reciprocal row.
"""

import math
import os
import sys

import numpy as np

for _p in ("/opt/trn_rl_repo", os.path.expanduser("~/.axon_site/_ro/trn_rl_repo")):
    if os.path.isdir(_p) and _p not in sys.path:
        sys.path.append(_p)

import ml_dtypes  # noqa: E402
import concourse.bass as bass  # noqa: E402
from concourse import bacc  # noqa: E402
import concourse.mybir as mybir  # noqa: E402
import concourse.tile as tile  # noqa: E402
from concourse import bass_utils  # noqa: E402

BF16NP = ml_dtypes.bfloat16

HIDDEN = 2048
NUM_HEADS = 32
NUM_KV_HEADS = 8
HEAD_DIM = 64
GROUPS = 4
SEQ = 2048
BATCH = 2
NCORES = 8
FH = 512  # features per core (8 q heads * 64)
PERM = [0, 4, 1, 5, 2, 6, 3, 7]  # local q-head order within a core

BF = mybir.dt.bfloat16
F32 = mybir.dt.float32

_CACHE = {}


# ----------------------------------------------------------------- host math
def _yarn_cos_sin():
    """Replicates reference._yarn_cos_sin for seq_len=SEQ. [SEQ, 32] f32."""
    dim = HEAD_DIM
    rope_base = 10000.0
    yarn_factor = 195.3
    max_seq = 4096
    pos_freqs = rope_base ** (np.arange(0, dim, 2, dtype=np.float64) / dim)
    inv_extra = 1.0 / pos_freqs
    inv_inter = 1.0 / (yarn_factor * pos_freqs)

    def corr_dim(num_rot):
        return (
            dim
            * math.log(max_seq / (num_rot * 2 * math.pi))
            / (2 * math.log(rope_base))
        )

    low = max(math.floor(corr_dim(32.0)), 0)
    high = min(math.ceil(corr_dim(1.0)), dim // 2 - 1)
    ramp = np.clip(
        (np.arange(dim // 2, dtype=np.float64) - low) / max(high - low, 1e-3), 0.0, 1.0
    )
    extrap = 1.0 - ramp
    inv_freq = inv_inter * (1.0 - extrap) + inv_extra * extrap
    t = np.arange(SEQ, dtype=np.float64)
    freqs = np.outer(t, inv_freq)
    mscale = 0.1 * math.log(yarn_factor) + 1.0
    cos = (np.cos(freqs) * mscale).astype(np.float32)
    sin = (np.sin(freqs) * mscale).astype(np.float32)
    return cos, sin


def _host_constants():
    cos, sin = _yarn_cos_sin()
    # expanded rope tables in feature-on-partition layout: row r <-> d = r%64
    idx = (np.arange(128) % 64) // 2
    cosE = np.ascontiguousarray(cos.T[idx, :]).astype(BF16NP)  # [128, SEQ]
    sinE = np.ascontiguousarray(sin.T[idx, :]).astype(BF16NP)

    # signed pair-swap permutation: rot = P.T @ q ; rot[2i] = -q[2i+1],
    # rot[2i+1] = q[2i]
    rotP = np.zeros((128, 128), dtype=BF16NP)
    for i in range(64):
        rotP[2 * i + 1, 2 * i] = -1.0
        rotP[2 * i, 2 * i + 1] = 1.0

    # row-selector for reciprocal broadcast: sel8[r, c] = (r == c // 64)
    sel8 = np.zeros((8, 512), dtype=np.float32)
    for h in range(8):
        sel8[h, h * 64 : (h + 1) * 64] = 1.0
    return cosE, sinE, rotP, sel8


# --------------------------------------------------------------- bass kernel
def _emit(tc, nc, aps):
    P = 128
    Exp = mybir.ActivationFunctionType.Exp
    mult = mybir.AluOpType.mult
    addop = mybir.AluOpType.add

    xT, wqT, wkT, wvT, woT, cosD, sinD, rotD, selD, yT = aps

    cst = tc.alloc_tile_pool(name="cst", bufs=1)
    big = tc.alloc_tile_pool(name="big", bufs=1)
    wts = tc.alloc_tile_pool(name="wts", bufs=1)
    tmp = tc.alloc_tile_pool(name="tmp", bufs=2)

    # ---- constants and inputs
    cos_sb = cst.tile([P, SEQ], BF)
    sin_sb = cst.tile([P, SEQ], BF)
    rot_sb = cst.tile([P, P], BF)
    sel_sb = cst.tile([8, 512], F32)
    nc.sync.dma_start(cos_sb, cosD)
    nc.sync.dma_start(sin_sb, sinD)
    nc.sync.dma_start(rot_sb, rotD)
    nc.sync.dma_start(sel_sb, selD)

    xT_sb = big.tile([P, 16, SEQ], BF)
    for k in range(16):
        nc.sync.dma_start(xT_sb[:, k, :], xT[k * P : (k + 1) * P, :])
    wq_sb = wts.tile([P, 16, FH], BF)
    wk_sb = wts.tile([P, 16, 128], BF)
    wv_sb = wts.tile([P, 16, 128], BF)
    for k in range(16):
        nc.sync.dma_start(wk_sb[:, k, :], wkT[k * P : (k + 1) * P, :])
        nc.sync.dma_start(wv_sb[:, k, :], wvT[k * P : (k + 1) * P, :])
        nc.sync.dma_start(wq_sb[:, k, :], wqT[k * P : (k + 1) * P, :])
    wo_sb = big.tile([P, 4, SEQ], BF)
    for k in range(4):
        nc.sync.dma_start(wo_sb[:, k, :], woT[k * P : (k + 1) * P, :])

    Qr_sb = big.tile([P, 4, SEQ], BF)  # rope'd Q^T, f-tile i = heads (i, i+4)
    Kr_sb = big.tile([P, SEQ], BF)  # rope'd K^T (kv0 rows 0:64, kv1 64:128)
    V_sb = big.tile([P, 16, 130], BF)  # [t-tile][kv0 64 | 1 | kv1 64 | 1]

    pp1 = tc.alloc_tile_pool(name="pp1", bufs=1, space="PSUM")

    def rope_chunk(dst, src_ps, j):
        jc = slice(j * 512, (j + 1) * 512)
        qtmp = tmp.tile([P, 512], BF, tag="qtmp", bufs=3)
        nc.vector.tensor_copy(qtmp, src_ps)
        rps = pp1.tile([P, 512], F32, tag="rops", bufs=2)
        nc.tensor.matmul(rps, rot_sb, qtmp, start=True, stop=True)
        m1 = tmp.tile([P, 512], BF, tag="m1", bufs=2)
        nc.vector.tensor_tensor(m1, qtmp, cos_sb[:, jc], op=mult)
        m2 = tmp.tile([P, 512], BF, tag="m2", bufs=2)
        nc.vector.tensor_tensor(m2, rps, sin_sb[:, jc], op=mult)
        nc.vector.tensor_tensor(dst, m1, m2, op=addop)

    # K projection + rope
    for j in range(4):
        kps = pp1.tile([P, 512], F32, tag="qps", bufs=3)
        for k in range(16):
            nc.tensor.matmul(
                kps,
                wk_sb[:, k, :],
                xT_sb[:, k, j * 512 : (j + 1) * 512],
                start=(k == 0),
                stop=(k == 15),
            )
        rope_chunk(Kr_sb[:, j * 512 : (j + 1) * 512], kps, j)

    # Q projection + rope
    for fi in range(4):
        for j in range(4):
            qps = pp1.tile([P, 512], F32, tag="qps", bufs=3)
            for k in range(16):
                nc.tensor.matmul(
                    qps,
                    wq_sb[:, k, fi * P : (fi + 1) * P],
                    xT_sb[:, k, j * 512 : (j + 1) * 512],
                    start=(k == 0),
                    stop=(k == 15),
                )
            rope_chunk(Qr_sb[:, fi, j * 512 : (j + 1) * 512], qps, j)

    # V projection (token-on-partition) + ones columns
    nc.vector.memset(V_sb, 1.0)
    for t in range(16):
        vps = pp1.tile([P, 128], F32, tag="vps", bufs=2)
        for k in range(16):
            nc.tensor.matmul(
                vps,
                xT_sb[:, k, t * P : (t + 1) * P],
                wv_sb[:, k, :],
                start=(k == 0),
                stop=(k == 15),
            )
        nc.vector.tensor_copy(V_sb[:, t, 0:64], vps[:, 0:64])
        nc.vector.tensor_copy(V_sb[:, t, 65:129], vps[:, 64:128])

    pp1.release()
    tmp.release()
    wts.release()

    tmp2 = tc.alloc_tile_pool(name="tmp2", bufs=2)
    ph2 = tc.alloc_tile_pool(name="ph2", bufs=1)

    # ---- phase 2: attention + output projection, per 512-token q chunk
    E_sb = big.tile([P, 16, 2, 512], BF)  # exp(S^T) [tk-tile][half][tq]
    OT_sb = big.tile([P, 4, SEQ], BF)  # normalized attn out, feature layout
    Oraw = ph2.tile([64, 8, 512], BF, bufs=1)
    den8 = ph2.tile([8, 512], F32, bufs=1)
    rec8 = ph2.tile([8, 512], F32, bufs=1)
    rscr = ph2.tile([8, 512], F32, bufs=1)

    dram = tc.alloc_tile_pool(name="dram", bufs=2, space="DRAM")

    pp2 = tc.alloc_tile_pool(name="pp2", bufs=1, space="PSUM")

    def oproj_tile(j, m):
        jc = slice(j * 512, (j + 1) * 512)
        yps = pp2.tile([P, 512], F32, tag="scr", bufs=2)
        for k2 in range(4):
            nc.tensor.matmul(
                yps,
                wo_sb[:, k2, m * P : (m + 1) * P],
                OT_sb[:, k2, jc],
                start=(k2 == 0),
                stop=(k2 == 3),
            )
        ysb = tmp2.tile([P, 512], F32, tag="ysb", bufs=3)
        if m % 2 == 0:
            nc.vector.tensor_copy(ysb, yps)
        else:
            nc.scalar.activation(ysb, yps, mybir.ActivationFunctionType.Copy)
        nc.sync.dma_start(yT[m * P : (m + 1) * P, jc], ysb)

    for j in range(4):
        jc = slice(j * 512, (j + 1) * 512)
        den_dram = dram.tile([8, 512], F32, tag="dend", bufs=2)
        for i in range(4):
            pavA = pp2.tile([65, 512], F32, tag="pav", bufs=2)
            pavB = pp2.tile([65, 512], F32, tag="pav", bufs=2)

            def av_step(k):
                for u, pav in ((0, pavA), (1, pavB)):
                    nc.tensor.matmul(
                        pav,
                        V_sb[:, k, u * 65 : (u + 1) * 65],
                        E_sb[:, k, u, :],
                        start=(k == 0),
                        stop=(k == 15),
                    )

            # software pipeline: AV for tile k-1 is emitted after QK/exp of
            # tile k, so the PE never sits in front of an exp it must wait on
            for k in range(16):
                S_t = pp2.tile([P, 1024], F32, tag="S", bufs=2)
                ks = slice(k * P, (k + 1) * P)
                nc.tensor.matmul(
                    S_t[:, 0:512], Kr_sb[0:64, ks], Qr_sb[0:64, i, jc],
                    start=True, stop=True,
                )
                nc.tensor.matmul(
                    S_t[:, 512:1024], Kr_sb[64:128, ks], Qr_sb[64:128, i, jc],
                    start=True, stop=True,
                )
                nc.scalar.activation(
                    E_sb[:, k, :, :],
                    S_t.rearrange("p (u c) -> p u c", c=512),
                    Exp,
                    scale=0.125,
                )
                if k >= 1:
                    av_step(k - 1)
                # interleave one o-proj tile of the previous chunk into the
                # PE stream so exp waits don't stall the array
                if j > 0 and k % 4 == 3:
                    oproj_tile(j - 1, i * 4 + k // 4)
            av_step(15)
            for u, pav in ((0, pavA), (1, pavB)):
                h = i + 4 * u
                # PSUM row 64 (denominator) -> SBUF lane 64 -> DRAM -> lanes 0-7
                den1 = tmp2.tile([P, 512], F32, tag="den1", bufs=2)
                nc.vector.tensor_copy(den1[64:65, :], pav[64:65, :])
                nc.sync.dma_start(den_dram[h : h + 1, :], den1[64:65, :])
                nc.vector.tensor_copy(Oraw[:, h, :], pav[0:64, :])
        nc.sync.dma_start(den8, den_dram)
        nc.vector.reciprocal_approx_accurate(rec8, den8, scratch=rscr)
        for h in range(8):
            i, u = h % 4, h // 4
            rps = pp2.tile([64, 512], F32, tag="scr", bufs=2)
            nc.tensor.matmul(
                rps, sel_sb[:, h * 64 : (h + 1) * 64], rec8, start=True, stop=True
            )
            if u == 0:
                nc.vector.tensor_tensor(
                    OT_sb[0:64, i, jc], Oraw[:, h, :], rps, op=mult
                )
            else:
                otmp = tmp2.tile([64, 512], BF, tag="otmp", bufs=2)
                nc.vector.tensor_tensor(otmp, Oraw[:, h, :], rps, op=mult)
                nc.sync.dma_start(OT_sb[64:128, i, jc], otmp)

    # trailing o-proj for the last chunk
    for m in range(16):
        oproj_tile(3, m)

    pp2.release()
    for p in (ph2, dram, tmp2, big, cst):
        p.release()


def _build():
    if "nc" in _CACHE:
        return _CACHE["nc"]
    nc = bacc.Bacc("TRN2", target_bir_lowering=False, debug=False, num_devices=NCORES)
    xT = nc.dram_tensor("xT", [HIDDEN, SEQ], BF, kind="ExternalInput").ap()
    wqT = nc.dram_tensor("wqT", [HIDDEN, FH], BF, kind="ExternalInput").ap()
    wkT = nc.dram_tensor("wkT", [HIDDEN, 128], BF, kind="ExternalInput").ap()
    wvT = nc.dram_tensor("wvT", [HIDDEN, 128], BF, kind="ExternalInput").ap()
    woT = nc.dram_tensor("woT", [FH, HIDDEN], BF, kind="ExternalInput").ap()
    cosD = nc.dram_tensor("cosE", [128, SEQ], BF, kind="ExternalInput").ap()
    sinD = nc.dram_tensor("sinE", [128, SEQ], BF, kind="ExternalInput").ap()
    rotD = nc.dram_tensor("rotP", [128, 128], BF, kind="ExternalInput").ap()
    selD = nc.dram_tensor("sel8", [8, 512], F32, kind="ExternalInput").ap()
    yT = nc.dram_tensor("yT", [HIDDEN, SEQ], F32, kind="ExternalOutput").ap()
    with tile.TileContext(nc) as tc:
        _emit(tc, nc, (xT, wqT, wkT, wvT, woT, cosD, sinD, rotD, selD, yT))
    nc.compile()
    _CACHE["nc"] = nc
    return nc


def _in_maps(hidden_states, wq, wk, wv, wo):
    cosE, sinE, rotP, sel8 = _host_constants()
    maps = []
    for c in range(NCORES):
        b, g = c // 4, c % 4
        feat = np.concatenate(
            [np.arange(64) + 64 * (8 * g + hl) for hl in PERM]
        )
        maps.append(
            {
                "xT": np.ascontiguousarray(hidden_states[b].T).astype(BF16NP),
                "wqT": np.ascontiguousarray(wq[feat, :].T).astype(BF16NP),
                "wkT": np.ascontiguousarray(
                    wk[128 * g : 128 * (g + 1), :].T
                ).astype(BF16NP),
                "wvT": np.ascontiguousarray(
                    wv[128 * g : 128 * (g + 1), :].T
                ).astype(BF16NP),
                "woT": np.ascontiguousarray(wo[:, feat].T).astype(BF16NP),
                "cosE": cosE,
                "sinE": sinE,
                "rotP": rotP,
                "sel8": sel8,
            }
        )
    return maps


def kernel(hidden_states, wq, wk, wv, wo):
    nc = _build()
    maps = _in_maps(
        np.asarray(hidden_states, dtype=np.float32),
        np.asarray(wq, dtype=np.float32),
        np.asarray(wk, dtype=np.float32),
        np.asarray(wv, dtype=np.float32),
        np.asarray(wo, dtype=np.float32),
    )
    res = bass_utils.run_bass_kernel_spmd(nc, maps, list(range(NCORES))).results
    y = np.zeros((BATCH, SEQ, HIDDEN), dtype=np.float64)
    for c in range(NCORES):
        y[c // 4] += res[c]["yT"].T.astype(np.float64)
    return y.astype(np.float32)
